# revision 1
# baseline (speedup 1.0000x reference)
"""Trainium2 Bass kernel for AxialSelfAttention2d.

Reference computation (per batch b):
    qkv = W @ x + b            (1x1 conv; W [3E, E], x [E, S, L], E = 512)
    q, k, v split; q *= Dh**-0.5; per head h: q,k,v [Dh=64, S, L]
    col:  scores[s,t|l] = q[:,s,l].k[:,t,l]; softmax over t; out_col = attn @ v
    row:  scores[l,m|s] = q[:,s,l].k[:,s,m]; softmax over m; out_row = attn @ v
    out = out_col + out_row    -> [H*Dh, S, L]

Sharding: 8 cores = 2 batches x 4 head-pairs. Each core computes 2 heads of
one batch end-to-end (no collectives); the host concatenates core outputs.

Per-core dataflow (matmul operands fp16, fp32 PSUM accumulation):
  A)  x fp32 --cast-DMA--> SBUF fp16 (half-group tiles, 3-deep prefetch);
      QKV projection with W^T stationary -> q2, k2 [128(2h x 64d), S*L].
      v lands in a 2-slab ring (16 s-rows per slab); as each slab completes
      it is DMA-transposed into vt_row[h][l, s16, sr, d] and gpsimd
      (l,s)-reordered into v_ls [hd, l*128+s], so stage A ends with all v
      layouts ready. PSUM evacuation (+bias; q pre-scaled on host) is split
      across DVE (tensor_scalar_add) and ACT (activation Identity with
      per-partition bias) to balance engines.
  B)  row attention per (s, h): scT[m,l'] = k_s^T @ q_s (PE, K=64, heads
      concurrent via base-partition row split); e = exp(scT) (ACT, no
      max-subtraction -- scores ~N(0,1)); AV: av[:, h, j*65+d] = e^T.T @
      vt_row slice plus an N=1 matmul against a ones column (same stationary
      weights) giving the softmax denominator at col j*65+64. Both heads
      share one 2-bank av tile (AV lhsT=et spans all partitions -> same
      tile_position), so one DVE reciprocal + one fused 4-dim-AP divide
      covers a whole quad -> src[l', X*128+hd]; DMA-transpose ->
      dst[hd, s*128+l] (final orientation).
  C)  col attention symmetric (vt_col built from v_ls by 16 packed
      DMA-transposes overlapping the row phase) -> src[s', X*128+hd];
      DMA-transpose -> tr[hd, lr*128+s']; strided adds merge into dst,
      split per 32-s block across DVE/gpsimd (last chunk all-DVE so the
      fp16 output DMA pipelines right behind it). Host casts fp16 -> fp32.
"""

import numpy as np

NUM_HEADS = 8
DIM_HEAD = 64
EMBED = 512
B, S, L = 2, 128, 128
SL = S * L
N_CORES = 8
HPC = 2  # heads per core

# Which engine evacuates each QKV projection output from PSUM.
# "dve" | "act" | "alt" (alternate by op index)
EVAC_PLAN = {"q": "dve", "k": "act", "v": "act"}

_CACHE = {}


def build_program(nc, tc):
    import concourse.bass as bass
    import concourse.mybir as mybir

    f16 = mybir.dt.float16
    f32 = mybir.dt.float32
    AF = mybir.ActivationFunctionType
    OP = mybir.AluOpType
    AP = bass.AP

    x_d = nc.dram_tensor("x", [EMBED, S, L], f32, kind="ExternalInput")
    w_d = nc.dram_tensor("wT", [EMBED, 384], f16, kind="ExternalInput")
    b_d = nc.dram_tensor("bvec", [384], f32, kind="ExternalInput")
    out_d = nc.dram_tensor("out", [128, S, L], f16, kind="ExternalOutput")

    x_flat = x_d.ap().rearrange("c s l -> c (s l)")
    out_flat = out_d.ap().rearrange("c s l -> c (s l)")

    GW = 2048         # spatial columns per x load (16 s-values)
    NG = SL // GW     # 8
    CH = 16           # i-values per attention output chunk
    NCH = 128 // CH   # 8

    import os
    stage = os.environ.get("AXIAL_DEBUG_STAGE", "full")

    def evac(kind, idx, dest_ap, ps_ap, bias_ap):
        plan = EVAC_PLAN[kind]
        use_act = plan == "act" or (plan == "alt" and idx % 2 == 0)
        if use_act:
            nc.scalar.activation(dest_ap, ps_ap, AF.Identity, bias=bias_ap)
        else:
            nc.vector.tensor_scalar_add(dest_ap, ps_ap, bias_ap)

    # ---------------- attention chunk (direction 0 = col, 1 = row) ----------
    def attention_chunk(direction, ch, vt, qv, kv, dst, zero_sb, ones_sb,
                        pools, i_start=None, width=None, last=False):
        st_pool, sc_ps, av_ps, e_pool, den_pool = pools
        if i_start is None:
            i_start = ch * CH
        if width is None:
            width = CH

        if direction == 0:
            def qk_slice(t, h, i):  # [64, t/s] column i, stride L
                return t[h * 64:(h + 1) * 64, :, i]
        else:
            def qk_slice(t, h, i):  # [64, m/l] row i, contiguous
                return t[h * 64:(h + 1) * 64, i, :]

        # PSUM-bank discipline: matmuls with different tile_positions must
        # never write the same bank (HW fault) -> per-head score/av banks.
        # Heads run concurrently on PE row-halves (lhsT base partition).
        src = st_pool.tile([128, CH * 128], f16, tag="st", name="src")
        for quad in range(width // 4):
            i0 = i_start + quad * 4
            # av[:, h, j*65 + {d, 64:denominator}] -- both heads share the
            # 2-bank tile (AV lhsT = et spans all partitions, so both heads
            # use tile_position (0,0); same-bank writes are safe).
            av = av_ps.tile([128, 2, 512], f32, tag="av", name="av")
            for h in range(2):
                sc = sc_ps.tile([128, 512], f32, tag=f"sc{h}", name="sc")
                for j in range(4):
                    nc.tensor.matmul(
                        sc[:][:, j * 128:(j + 1) * 128],
                        qk_slice(kv, h, i0 + j),
                        qk_slice(qv, h, i0 + j),
                        start=True, stop=True)
                et = e_pool.tile([128, 512], f16, tag="et", name="et")
                nc.scalar.activation(et[:], sc[:], AF.Exp,
                                     bias=zero_sb[:][:, 0:1])
                for j in range(4):
                    i = i0 + j
                    nc.tensor.matmul(
                        av[:][:, h, j * 65:j * 65 + 64],
                        et[:][:, j * 128:(j + 1) * 128],
                        vt[h][:][:, i // 16, i % 16, :],
                        start=True, stop=True)
                    nc.tensor.matmul(
                        av[:][:, h, j * 65 + 64:j * 65 + 65],
                        et[:][:, j * 128:(j + 1) * 128],
                        ones_sb[:][:, 0:1],
                        start=True, stop=True)
            # den[p, j*2+h] = 1 / av[p, h, j*65+64]
            den = den_pool.tile([128, 8], f32, tag="den", name="den")
            nc.vector.reciprocal(
                den[:], AP(av[:].tensor, av[:].offset + 64,
                           [list(av[:].ap[0]), [65, 4], [512, 2]]))
            # src[p, (quad*4+j)*128 + h*64 + d] = av[:, h, j*65+d]*den[:, 2j+h]
            in0 = AP(av[:].tensor, av[:].offset,
                     [list(av[:].ap[0]), [65, 4], [512, 2], [1, 64]])
            in1 = AP(den[:].tensor, den[:].offset,
                     [list(den[:].ap[0]), [2, 4], [1, 2], [0, 64]])
            o = AP(src[:].tensor, src[:].offset + (quad * 4) * 128,
                   [list(src[:].ap[0]), [128, 4], [64, 2], [1, 64]])
            nc.vector.tensor_tensor(o, in0, in1, OP.mult)

        if direction == 1:
            # row: transpose lands directly in dst
            # dst[hd, (ch*CH+X)*128 + l'] <- src[l', X*128+hd]
            od = AP(dst[:].tensor, dst[:].offset + i_start * 128,
                    [list(dst[:].ap[0]), [128, width], [1, 128]])
            nc.sync.dma_start(od, src[:][:, 0:width * 128], transpose=True)
        else:
            # col: transpose to tr[hd, lr*128 + s'], then strided adds
            # dst[hd, s'*128 + ch*CH + lr] += tr[hd, lr*128 + s']
            # split by 32-s block; last chunk on DVE so the out DMA can
            # pipeline immediately behind each block's final merge.
            tr = st_pool.tile([128, CH * 128], f16, tag="st", name="tr")
            ot = AP(tr[:].tensor, tr[:].offset,
                    [list(tr[:].ap[0]), [128, width], [1, 128]])
            nc.sync.dma_start(ot, src[:][:, 0:width * 128], transpose=True)
            for sb in range(4):
                dseg = AP(dst[:].tensor,
                          dst[:].offset + sb * 32 * 128 + i_start,
                          [list(dst[:].ap[0]), [1, width], [128, 32]])
                tin = AP(tr[:].tensor, tr[:].offset + sb * 32,
                         [list(tr[:].ap[0]), [128, width], [1, 32]])
                if last or sb % 2 == 0:
                    nc.vector.tensor_tensor(dseg, dseg, tin, OP.add)
                else:
                    nc.gpsimd.tensor_tensor(dseg, dseg, tin, OP.add)

    # ---------------- top-level pools ----------------
    with tc.tile_pool(name="base", bufs=1) as base_pool, \
         tc.tile_pool(name="vtrp", bufs=1) as vtr_pool, \
         tc.tile_pool(name="vtcp", bufs=1) as vtc_pool, \
         tc.tile_pool(name="vlsp", bufs=1) as vls_pool:
        q2 = base_pool.tile([128, SL], f16, tag="q2")
        k2 = base_pool.tile([128, SL], f16, tag="k2")
        zero_sb = base_pool.tile([128, 1], f32, tag="z")
        nc.vector.memset(zero_sb[:], 0.0)
        ones_sb = base_pool.tile([128, 1], f16, tag="one")
        nc.vector.memset(ones_sb[:], 1.0)
        vtr = [vtr_pool.tile([128, 8, 16, 64], f16, tag=f"vtr{h}",
                             name=f"vtr{h}") for h in range(HPC)]
        vtc = [vtc_pool.tile([128, 8, 16, 64], f16, tag=f"vtc{h}",
                             name=f"vtc{h}") for h in range(HPC)]
        v_ls = vls_pool.tile([128, SL], f16, tag="v_ls")  # [hd, l*128+s]

        qv = q2[:].rearrange("p (s l) -> p s l", l=L)
        kv = k2[:].rearrange("p (s l) -> p s l", l=L)

        # ---- stage A ----
        # v lands in a 2-slab ring (32 s-rows per slab); as each slab
        # completes it is (a) DMA-transposed into vt_row and (b) gpsimd
        # (l,s)-reordered into v_ls, so nothing big blocks post-A.
        with tc.tile_pool(name="vring", bufs=2) as vring, \
             tc.tile_pool(name="xload", bufs=2) as xpool, \
             tc.tile_pool(name="qkvps", bufs=4, space="PSUM") as qkv_ps:
            w_sb = xpool.tile([128, 4, 384], f16, tag="w", bufs=1)
            nc.sync.dma_start(
                w_sb[:], w_d.ap().rearrange("(k c) o -> c k o", k=4))
            b_sb = xpool.tile([128, 3], f32, tag="b", bufs=1)
            nc.sync.dma_start(
                b_sb[:], b_d.ap().rearrange("(m p) -> p m", p=128))
            for g in range(NG):
                slab = vring.tile([128, 2048], f16, tag="vslab",
                                  name="vslab")
                xts = []
                for qtr in range(4):
                    xt = xpool.tile([128, 4, GW // 4], f16, tag="x",
                                    bufs=9, name="xt")
                    lo = g * GW + qtr * (GW // 4)
                    nc.gpsimd.dma_start(
                        xt[:],
                        x_flat[:, lo:lo + GW // 4]
                            .rearrange("(k c) n -> c k n", k=4))
                    xts.append(xt)
                for m in range(3):  # 0=q, 1=k, 2=v
                    dest = (q2, k2, slab)[m]
                    kind = "qkv"[m]
                    for sg in range(GW // 512):
                        xt = xts[sg]
                        xo = 0
                        ps = qkv_ps.tile([128, 512], f32, tag="acc",
                                         name="ps")
                        for c in range(4):
                            nc.tensor.matmul(
                                ps[:],
                                w_sb[:][:, c, m * 128:(m + 1) * 128],
                                xt[:][:, c, xo:xo + 512],
                                start=(c == 0), stop=(c == 3))
                        off = g * GW + sg * 512 if m < 2 else sg * 512
                        evac(kind, g * 4 + sg,
                             dest[:][:, off:off + 512], ps[:],
                             b_sb[:][:, m:m + 1])
                # slab for s in [16*g, 16*g+16) complete
                for h in range(HPC):
                    nc.sync.dma_start(
                        vtr[h][:][:, g],
                        slab[:][h * 64:(h + 1) * 64, :],
                        transpose=True)
                # v_ls[p, l*128 + g*16 + sr] = slab[p, sr*128 + l]
                vdst = AP(v_ls[:].tensor, v_ls[:].offset + g * 16,
                          [list(v_ls[:].ap[0]), [128, 128], [1, 16]])
                vsrc = AP(slab[:].tensor, slab[:].offset,
                          [list(slab[:].ap[0]), [1, 128], [128, 16]])
                nc.gpsimd.tensor_copy(vdst, vsrc)

        # ---- attention ----
        with tc.tile_pool(name="dstp", bufs=1) as dst_pool, \
             tc.tile_pool(name="stp", bufs=3) as st_pool, \
             tc.tile_pool(name="ep", bufs=3) as e_pool, \
             tc.tile_pool(name="denp", bufs=4) as den_pool, \
             tc.tile_pool(name="scps", bufs=2, space="PSUM") as sc_ps, \
             tc.tile_pool(name="avps", bufs=2, space="PSUM") as av_ps:
            dst = dst_pool.tile([128, SL], f16, tag="dst")  # [hd, s*128+l]
            pools = (st_pool, sc_ps, av_ps, e_pool, den_pool)

            def vtc_oc(oc):
                for h in range(HPC):
                    nc.sync.dma_start(
                        vtc[h][:][:, oc],
                        v_ls[:][h * 64:(h + 1) * 64,
                                oc * 2048:(oc + 1) * 2048],
                        transpose=True)

            if stage in ("row", "full"):
                for ch in range(NCH):
                    attention_chunk(1, ch, vtr, qv, kv, dst, zero_sb,
                                    ones_sb, pools)
                    # spread vt_col transpose issues between row chunks so
                    # SP's serial issue cost never delays a row transpose
                    vtc_oc(ch)
            else:
                for oc in range(8):
                    vtc_oc(oc)
            if stage == "full":
                for ch in range(NCH - 1):
                    attention_chunk(0, ch, vtc, qv, kv, dst, zero_sb,
                                    ones_sb, pools)
                base_i = (NCH - 1) * CH
                attention_chunk(0, NCH - 1, vtc, qv, kv, dst, zero_sb,
                                ones_sb, pools, i_start=base_i, width=8)
                attention_chunk(0, NCH - 1, vtc, qv, kv, dst, zero_sb,
                                ones_sb, pools, i_start=base_i + 8, width=8,
                                last=True)
            if stage == "a":
                nc.vector.tensor_copy(dst[:], q2[:])
            for sb in range(4):
                nc.sync.dma_start(
                    out_flat[:, sb * 32 * 128:(sb + 1) * 32 * 128],
                    dst[:][:, sb * 32 * 128:(sb + 1) * 32 * 128])


def _get_nc():
    if "nc" in _CACHE:
        return _CACHE["nc"]
    import concourse.bacc as bacc
    import concourse.tile as tile

    nc = bacc.Bacc(None, target_bir_lowering=False, debug=False,
                   num_devices=N_CORES)
    with tile.TileContext(nc) as tc:
        build_program(nc, tc)
    nc.compile()
    _CACHE["nc"] = nc
    return nc


def make_in_maps(x, W, b):
    x = np.asarray(x, dtype=np.float32)
    W = np.asarray(W, dtype=np.float32)
    b = np.asarray(b, dtype=np.float32)
    scale = np.float32(DIM_HEAD ** -0.5)
    in_maps = []
    for c in range(N_CORES):
        bb, h0 = c // 4, 2 * (c % 4)
        hd = np.arange(h0 * 64, (h0 + 2) * 64)
        sel = np.concatenate([hd, EMBED + hd, 2 * EMBED + hd])
        W_loc = W[sel, :].copy()
        b_loc = b[sel].copy()
        W_loc[:128] *= scale
        b_loc[:128] *= scale
        in_maps.append({
            "x": np.ascontiguousarray(x[bb]),
            "wT": np.ascontiguousarray(W_loc.T).astype(np.float16),
            "bvec": b_loc.astype(np.float32),
        })
    return in_maps


def assemble(results):
    out = np.empty((B, EMBED, S, L), dtype=np.float32)
    for c, r in enumerate(results):
        bb, h0 = c // 4, 2 * (c % 4)
        out[bb, h0 * 64:(h0 + 2) * 64] = r["out"].astype(np.float32)
    return out


def kernel(x, W, b):
    from concourse.bass_utils import run_bass_kernel_spmd
    nc = _get_nc()
    res = run_bass_kernel_spmd(nc, make_in_maps(x, W, b),
                               core_ids=list(range(N_CORES)))
    return assemble(res.results)



# revision 54
# speedup vs baseline: 1.0114x; 1.0114x over previous
"""Trainium2 Bass kernel for AxialSelfAttention2d (v3).

Reference computation (per batch b):
    qkv = W @ x + b            (1x1 conv; W [3E, E], x [E, S, L], E = 512)
    q, k, v split; q *= Dh**-0.5; per head h: q,k,v [Dh=64, S, L]
    col:  scores[s,t|l] = q[:,s,l].k[:,t,l]; softmax over t; out_col = attn @ v
    row:  scores[l,m|s] = q[:,s,l].k[:,s,m]; softmax over m; out_row = attn @ v
    out = out_col + out_row    -> [H*Dh, S, L]

Sharding: 8 cores = 2 batches x 4 head-pairs. Each core computes 2 heads of
one batch end-to-end (no collectives); the host concatenates core outputs.

Per-core dataflow (matmul operands fp16, fp32 PSUM):
  A)  x shipped fp16 from host (halves HBM read); s-major slabs of 16 rows.
      q,k projection with W^T stationary -> q2,k2 [128(2h x 64d), S*L],
      evacuated on ACT (identity+bias). v projection FLIPPED (x-tile
      stationary, Wv^T moving) so v lands already transposed:
      vtr[l, hd*128+s]; evac adds the (host-broadcast) v bias on DVE.
      ROW attention for slabs 0-5 interleaved behind each slab's
      projections (scores ~N(0,1): exp without max subtraction; softmax
      denominator via an extra ones-column matmul sharing the et
      stationary). One reciprocal + fused 4-dim-AP divide per quad ->
      src[l', X*128+hd], xbar-transposed straight into dst[hd, s*128+l].
  B)  vtc[s, hd*128+l] = 8 xbar transposes of vtr (the hd-OUTER vtr
      layout makes the s<->l swap a single transpose family); rows 6-7
      (with [128,1024] fused-2-head scores/exp) fill the transpose
      window.  COL attention symmetric; col src[s', X*128+hd] is
      xbar-transposed into a small l-major ring tile dc[hd, X*128+s'],
      one strided add folds the row result in (dc += dst slice), and
      out[l-chunk] DMAs straight from dc -- fully per-chunk pipelined,
      no end-of-kernel merge tail.  Engine roles keep DMA-dependent ops
      off the DVE/ACT queues (in-order SEQs hold during waits).
  Host swaps (l,s)->(s,l) and casts fp16 -> fp32.
"""

import numpy as np

NUM_HEADS = 8
DIM_HEAD = 64
EMBED = 512
B, S, L = 2, 128, 128
SL = S * L
N_CORES = 8
HPC = 2  # heads per core

_CACHE = {}


def build_program(nc, tc):
    import concourse.bass as bass
    import concourse.mybir as mybir

    f16 = mybir.dt.float16
    f32 = mybir.dt.float32
    AF = mybir.ActivationFunctionType
    OP = mybir.AluOpType
    AP = bass.AP

    x_d = nc.dram_tensor("x", [EMBED, S, L], f16, kind="ExternalInput")
    w_d = nc.dram_tensor("wT", [EMBED, 384], f16, kind="ExternalInput")
    b_d = nc.dram_tensor("bvec", [384], f32, kind="ExternalInput")
    bvt_d = nc.dram_tensor("bvt", [128, 512], f32, kind="ExternalInput")
    id_d = nc.dram_tensor("ident", [128, 128], f16, kind="ExternalInput")
    # l-major output: out[hd, l, s]; host swaps back to [hd, s, l]
    out_d = nc.dram_tensor("out", [128, L, S], f16, kind="ExternalOutput")

    x_flat = x_d.ap().rearrange("c s l -> c (s l)")
    out_flat = out_d.ap().rearrange("c l s -> c (l s)")

    GW = 2048         # spatial columns per slab (16 s-rows)
    NG = SL // GW     # 8
    CH = 16           # i-values per attention chunk
    NCH = 128 // CH   # 8
    NROW_A = 6        # row chunks interleaved into stage A

    # ---------------- top-level tiles ----------------
    with tc.tile_pool(name="base", bufs=1) as base_pool, \
         tc.tile_pool(name="etp", bufs=3) as e_pool, \
         tc.tile_pool(name="srcp", bufs=2) as src_pool, \
         tc.tile_pool(name="denp", bufs=4) as den_pool, \
         tc.tile_pool(name="avr", bufs=2, space="PSUM") as av_pool:
        q2 = base_pool.tile([128, SL], f16, tag="q2")
        k2 = base_pool.tile([128, SL], f16, tag="k2")
        vtr = base_pool.tile([128, SL], f16, tag="vtr")  # [l, hd*128+s]
        vtc = base_pool.tile([128, SL], f16, tag="vtc")  # [s, hd*128+l]
        dst = base_pool.tile([128, SL], f16, tag="dst")  # [hd, s*128+l]
        zero_sb = base_pool.tile([128, 1], f32, tag="z")
        nc.vector.memset(zero_sb[:], 0.0)
        ones_sb = base_pool.tile([128, 1], f16, tag="one")
        nc.vector.memset(ones_sb[:], 1.0)
        ident = base_pool.tile([128, 128], f16, tag="id")
        nc.sync.dma_start(ident[:], id_d.ap())
        bvt_sb = base_pool.tile([128, 512], f32, tag="bvt")
        nc.sync.dma_start(bvt_sb[:], bvt_d.ap())

        qv = q2[:].rearrange("p (s l) -> p s l", l=L)
        kv = k2[:].rearrange("p (s l) -> p s l", l=L)

        def evac_qk(eng, dest_ap, ps_ap, bias_ap):
            if eng == "act":
                nc.scalar.activation(dest_ap, ps_ap, AF.Identity, bias=bias_ap)
            elif eng == "dve":
                nc.vector.tensor_scalar_add(dest_ap, ps_ap, bias_ap)
            else:
                nc.gpsimd.tensor_scalar_add(dest_ap, ps_ap, bias_ap)

        # ---------------- attention chunk ----------------
        # direction 0 = col (i over l), 1 = row (i over s).
        # sc_tile(quad) -> (sc_ap_fn(h, j), exp_fn(ets_out)) abstraction:
        # small mode: per-head [128,512] tiles; big mode: one [128,1024].
        def attention_chunk(direction, ch, vt, sc_alloc):
            i0 = ch * CH

            if direction == 0:
                def qk_slice(t, h, i):
                    return t[h * 64:(h + 1) * 64, :, i]
            else:
                def qk_slice(t, h, i):
                    return t[h * 64:(h + 1) * 64, i, :]

            def vt_slice(h, i):
                return AP(vt.tensor, vt.offset + h * 64 * 128 + i,
                          [list(vt.ap[0]), [128, 64]])

            src = src_pool.tile([128, CH * 128], f16, tag="st", name="src")
            pend = None

            def issue_av(work):
                quad, et_slc, av = work
                for h in range(HPC):
                    for j in range(4):
                        i = i0 + quad * 4 + j
                        nc.tensor.matmul(
                            av[:][:, h, j * 65:j * 65 + 64],
                            et_slc(h, j),
                            vt_slice(h, i),
                            start=True, stop=True)
                        nc.tensor.matmul(
                            av[:][:, h, j * 65 + 64:j * 65 + 65],
                            et_slc(h, j),
                            ones_sb[:][:, 0:1],
                            start=True, stop=True)
                den = den_pool.tile([128, 8], f32, tag="den", name="den")
                nc.vector.reciprocal(
                    den[:], AP(av[:].tensor, av[:].offset + 64,
                               [list(av[:].ap[0]), [65, 4], [512, 2]]))
                in0 = AP(av[:].tensor, av[:].offset,
                         [list(av[:].ap[0]), [65, 4], [512, 2], [1, 64]])
                in1 = AP(den[:].tensor, den[:].offset,
                         [list(den[:].ap[0]), [2, 4], [1, 2], [0, 64]])
                o = AP(src[:].tensor, src[:].offset + (quad * 4) * 128,
                       [list(src[:].ap[0]), [128, 4], [64, 2], [1, 64]])
                nc.vector.tensor_tensor(o, in0, in1, OP.mult)

            for quad in range(4):
                et_slc = sc_alloc(quad, i0, qk_slice)
                av = av_pool.tile([128, 2, 512], f32, tag="av", name="av")
                if pend is not None:
                    issue_av(pend)
                pend = (quad, et_slc, av)
            issue_av(pend)

            if direction == 1:
                # row result straight to dst[hd, (i0+X)*128 + l']
                od = AP(dst[:].tensor, dst[:].offset + i0 * 128,
                        [list(dst[:].ap[0]), [128, CH], [1, 128]])
                nc.sync.dma_start(od, src[:], transpose=True)
            return src

        def small_sc_alloc(pool):
            def alloc(quad, i0, qk_slice):
                ets = []
                for h in range(HPC):
                    sc = pool.tile([128, 512], f32, tag="ps", name="sc")
                    for j in range(4):
                        i = i0 + quad * 4 + j
                        nc.tensor.matmul(
                            sc[:][:, j * 128:(j + 1) * 128],
                            qk_slice(kv, h, i),
                            qk_slice(qv, h, i),
                            start=True, stop=True)
                    et = e_pool.tile([128, 512], f16, tag="et", name="et")
                    nc.scalar.activation(et[:], sc[:], AF.Exp,
                                         bias=zero_sb[:][:, 0:1])
                    ets.append(et)
                return lambda h, j: ets[h][:][:, j * 128:(j + 1) * 128]
            return alloc

        def big_sc_alloc(pool):
            def alloc(quad, i0, qk_slice):
                sc = pool.tile([128, 1024], f32, tag="cs", name="sc")
                for h in range(HPC):
                    for j in range(4):
                        i = i0 + quad * 4 + j
                        nc.tensor.matmul(
                            sc[:][:, h * 512 + j * 128:
                                  h * 512 + (j + 1) * 128],
                            qk_slice(kv, h, i),
                            qk_slice(qv, h, i),
                            start=True, stop=True)
                et = e_pool.tile([128, 1024], f16, tag="et", name="et")
                nc.scalar.activation(et[:], sc[:], AF.Exp,
                                     bias=zero_sb[:][:, 0:1])
                return lambda h, j: et[:][:, h * 512 + j * 128:
                                          h * 512 + (j + 1) * 128]
            return alloc

        # ---- col chunk finish: xbar-transpose into a per-chunk l-major
        # tile, fold the row result in with one strided add, DMA out ----
        def col_transpose(src):
            # src[s', X*128+hd] --xbar--> dc[hd, X*128+s']
            dc = src_pool.tile([128, CH * 128], f16, tag="dc", name="dc",
                               bufs=3)
            ot = AP(dc[:].tensor, dc[:].offset,
                    [list(dc[:].ap[0]), [128, CH], [1, 128]])
            nc.sync.dma_start(ot, src[:], transpose=True)
            return dc

        def col_add(ch, dc, eng):
            # dc[hd, X*128+s'] += dst[hd, s'*128 + (i0+X)]  (row part)
            i0 = ch * CH
            do = AP(dc[:].tensor, dc[:].offset,
                    [list(dc[:].ap[0]), [128, CH], [1, 128]])
            ri = AP(dst[:].tensor, dst[:].offset + i0,
                    [list(dst[:].ap[0]), [1, CH], [128, 128]])
            eng.tensor_tensor(do, do, ri, OP.add)

        def col_out(ch, dc, eng=None):
            i0 = ch * CH
            (eng or nc.sync).dma_start(
                out_flat[:, i0 * 128:(i0 + CH) * 128], dc[:])

        # ---------------- stage A ----------------
        with tc.tile_pool(name="psa", bufs=4, space="PSUM") as psa, \
             tc.tile_pool(name="xload", bufs=2) as xpool:
            small_alloc = small_sc_alloc(psa)
            w_sb = xpool.tile([128, 4, 384], f16, tag="w", bufs=1)
            nc.sync.dma_start(
                w_sb[:], w_d.ap().rearrange("(k c) o -> c k o", k=4))
            b_sb = xpool.tile([128, 3], f32, tag="b", bufs=1)
            nc.sync.dma_start(
                b_sb[:], b_d.ap().rearrange("(m p) -> p m", p=128))
            for g in range(NG):
                xts = []
                for qtr in range(4):
                    xt = xpool.tile([128, 4, GW // 4], f16, tag="x",
                                    bufs=4, name="xt")
                    lo = g * GW + qtr * (GW // 4)
                    nc.sync.dma_start(
                        xt[:],
                        x_flat[:, lo:lo + GW // 4]
                            .rearrange("(k c) n -> c k n", k=4))
                    xts.append(xt)
                # q, k projections: W stationary, x moving
                for m in range(2):
                    dest = (q2, k2)[m]
                    eng_rot = ("act", "act", "act", "act")
                    for sg in range(4):
                        ps = psa.tile([128, 512], f32, tag="ps", name="ps")
                        for c in range(4):
                            nc.tensor.matmul(
                                ps[:],
                                w_sb[:][:, c, m * 128:(m + 1) * 128],
                                xts[sg][:][:, c, :],
                                start=(c == 0), stop=(c == 3))
                        off = g * GW + sg * 512
                        evac_qk(eng_rot[sg], dest[:][:, off:off + 512],
                                ps[:], b_sb[:][:, m:m + 1])
                # v projection flipped: x-tile stationary, Wv^T moving
                for sg in range(4):
                    vt_ps = psa.tile([128, 512], f32, tag="ps", name="vt")
                    for sr in range(4):
                        for c in range(4):
                            nc.tensor.matmul(
                                vt_ps[:][:, sr * 128:(sr + 1) * 128],
                                xts[sg][:][:, c, sr * 128:(sr + 1) * 128],
                                w_sb[:][:, c, 256:384],
                                start=(c == 0), stop=(c == 3))
                    # vtr[l, hd*128 + (s0+j)] = vt_ps[l, j*128+hd] + b_v
                    s0 = g * 16 + sg * 4
                    vo = AP(vtr[:].tensor, vtr[:].offset + s0,
                            [list(vtr[:].ap[0]), [128, 128], [1, 4]])
                    vi = AP(vt_ps[:].tensor, vt_ps[:].offset,
                            [list(vt_ps[:].ap[0]), [1, 128], [128, 4]])
                    bi = AP(bvt_sb[:].tensor, bvt_sb[:].offset,
                            [list(bvt_sb[:].ap[0]), [1, 128], [128, 4]])
                    nc.vector.tensor_tensor(vo, vi, bi, OP.add)
                import os
                if g < NROW_A and os.environ.get("AXIAL_STAGE") != "col":
                    attention_chunk(1, g, vtr, small_alloc)

        # ---------------- col phase ----------------
        import os
        stage = os.environ.get("AXIAL_STAGE", "full")
        with tc.tile_pool(name="psb", bufs=2, space="PSUM") as psb:
            big_alloc = big_sc_alloc(psb)
            # vtc[s, (16t+a)*128 + l] <- vtr[l, (16t+a)*128 + s]
            def vtc_issue(ts):
                for t in ts:
                    vo = AP(vtc[:].tensor, vtc[:].offset + t * 2048,
                            [list(vtc[:].ap[0]), [128, 16], [1, 128]])
                    nc.sync.dma_start(vo,
                                      vtr[:][:, t * 2048:(t + 1) * 2048],
                                      transpose=True)
            vtc_issue(range(0, 8))
            # remaining row chunks fill the vtc-transpose window
            if stage == "col":
                nc.vector.memset(dst[:], 0.0)
            else:
                for g in range(NROW_A, NG):
                    attention_chunk(1, g, vtr, big_alloc)
            if stage == "row":
                for sb in range(8):
                    nc.sync.dma_start(
                        out_flat[:, sb * 2048:(sb + 1) * 2048],
                        dst[:][:, sb * 2048:(sb + 1) * 2048])
                return
            # software-pipelined finish, staged so every op's wait is
            # short when its queue reaches it: trC(ch-1) + Pool-half
            # merge(ch-1) behind chunk ch; DVE-half merge(ch-2) (its trC
            # is certainly done -> the DVE queue never waits on DMA);
            # out(ch-3) on SP.
            srcs, dcs = {}, {}
            LAST = NCH - 1  # last chunk finishes eagerly (DVE add)

            def finish_steps(ch):
                if 1 <= ch and ch - 1 < LAST and (ch - 1) not in dcs:
                    dcs[ch - 1] = col_transpose(srcs[ch - 1])
                if 2 <= ch and ch - 2 < LAST:
                    col_add(ch - 2, dcs[ch - 2], nc.vector)
                if 3 <= ch and ch - 3 < LAST:
                    col_out(ch - 3, dcs[ch - 3])

            for ch in range(NCH):
                srcs[ch] = attention_chunk(0, ch, vtc, big_alloc)
                finish_steps(ch)
                if ch >= LAST:
                    # eager tail: transpose -> Pool add -> ACT-issued out
                    dcs[ch] = col_transpose(srcs[ch])
                    col_add(ch, dcs[ch], nc.vector)
                    col_out(ch, dcs[ch], nc.scalar)
            for ch in range(NCH, NCH + 3):
                finish_steps(ch)


def _get_nc():
    if "nc" in _CACHE:
        return _CACHE["nc"]
    import concourse.bacc as bacc
    import concourse.tile as tile

    nc = bacc.Bacc(None, target_bir_lowering=False, debug=False,
                   num_devices=N_CORES)
    with tile.TileContext(nc) as tc:
        build_program(nc, tc)
    nc.compile()
    _CACHE["nc"] = nc
    return nc


def make_in_maps(x, W, b):
    x = np.asarray(x, dtype=np.float32)
    W = np.asarray(W, dtype=np.float32)
    b = np.asarray(b, dtype=np.float32)
    scale = np.float32(DIM_HEAD ** -0.5)
    ident = np.eye(128, dtype=np.float16)
    in_maps = []
    for c in range(N_CORES):
        bb, h0 = c // 4, 2 * (c % 4)
        hd = np.arange(h0 * 64, (h0 + 2) * 64)
        sel = np.concatenate([hd, EMBED + hd, 2 * EMBED + hd])
        W_loc = W[sel, :].copy()
        b_loc = b[sel].copy()
        W_loc[:128] *= scale
        b_loc[:128] *= scale
        bvt = np.tile(b_loc[256:384], 4)[None, :].repeat(128, 0)
        in_maps.append({
            "x": np.ascontiguousarray(x[bb]).astype(np.float16),
            "wT": np.ascontiguousarray(W_loc.T).astype(np.float16),
            "bvec": b_loc.astype(np.float32),
            "bvt": np.ascontiguousarray(bvt).astype(np.float32),
            "ident": ident,
        })
    return in_maps


def assemble(results):
    out = np.empty((B, EMBED, S, L), dtype=np.float32)
    for c, r in enumerate(results):
        bb, h0 = c // 4, 2 * (c % 4)
        # device output is [hd, l, s]; swap back to [hd, s, l]
        out[bb, h0 * 64:(h0 + 2) * 64] = \
            np.swapaxes(r["out"], 1, 2).astype(np.float32)
    return out


def kernel(x, W, b):
    from concourse.bass_utils import run_bass_kernel_spmd
    nc = _get_nc()
    res = run_bass_kernel_spmd(nc, make_in_maps(x, W, b),
                               core_ids=list(range(N_CORES)))
    return assemble(res.results)


# revision 55
# speedup vs baseline: 1.0161x; 1.0046x over previous
"""Trainium2 Bass kernel for AxialSelfAttention2d (v3).

Reference computation (per batch b):
    qkv = W @ x + b            (1x1 conv; W [3E, E], x [E, S, L], E = 512)
    q, k, v split; q *= Dh**-0.5; per head h: q,k,v [Dh=64, S, L]
    col:  scores[s,t|l] = q[:,s,l].k[:,t,l]; softmax over t; out_col = attn @ v
    row:  scores[l,m|s] = q[:,s,l].k[:,s,m]; softmax over m; out_row = attn @ v
    out = out_col + out_row    -> [H*Dh, S, L]

Sharding: 8 cores = 2 batches x 4 head-pairs. Each core computes 2 heads of
one batch end-to-end (no collectives); the host concatenates core outputs.

Per-core dataflow (matmul operands fp16, fp32 PSUM):
  A)  x shipped fp16 from host (halves HBM read); s-major slabs of 16 rows.
      q,k projection with W^T stationary -> q2,k2 [128(2h x 64d), S*L],
      evacuated on ACT (identity+bias). v projection FLIPPED (x-tile
      stationary, Wv^T moving) so v lands already transposed:
      vtr[l, hd*128+s]; evac adds the (host-broadcast) v bias on DVE.
      ROW attention for slabs 0-5 interleaved behind each slab's
      projections (scores ~N(0,1): exp without max subtraction; softmax
      denominator via an extra ones-column matmul sharing the et
      stationary). One reciprocal + fused 4-dim-AP divide per quad ->
      src[l', X*128+hd], xbar-transposed straight into dst[hd, s*128+l].
  B)  vtc[s, hd*128+l] = 8 xbar transposes of vtr (the hd-OUTER vtr
      layout makes the s<->l swap a single transpose family); rows 6-7
      (with [128,1024] fused-2-head scores/exp) fill the transpose
      window.  COL attention symmetric; col src[s', X*128+hd] is
      xbar-transposed into a small l-major ring tile dc[hd, X*128+s'],
      one strided add folds the row result in (dc += dst slice), and
      out[l-chunk] DMAs straight from dc -- fully per-chunk pipelined,
      no end-of-kernel merge tail.  Engine roles keep DMA-dependent ops
      off the DVE/ACT queues (in-order SEQs hold during waits).
  Host swaps (l,s)->(s,l) and casts fp16 -> fp32.
"""

import numpy as np

NUM_HEADS = 8
DIM_HEAD = 64
EMBED = 512
B, S, L = 2, 128, 128
SL = S * L
N_CORES = 8
HPC = 2  # heads per core

_CACHE = {}


def build_program(nc, tc):
    import concourse.bass as bass
    import concourse.mybir as mybir

    f16 = mybir.dt.float16
    f32 = mybir.dt.float32
    AF = mybir.ActivationFunctionType
    OP = mybir.AluOpType
    AP = bass.AP

    x_d = nc.dram_tensor("x", [EMBED, S, L], f16, kind="ExternalInput")
    w_d = nc.dram_tensor("wT", [EMBED, 384], f16, kind="ExternalInput")
    b_d = nc.dram_tensor("bvec", [384], f32, kind="ExternalInput")
    bvt_d = nc.dram_tensor("bvt", [128, 512], f32, kind="ExternalInput")
    id_d = nc.dram_tensor("ident", [128, 128], f16, kind="ExternalInput")
    # l-major output: out[hd, l, s]; host swaps back to [hd, s, l]
    out_d = nc.dram_tensor("out", [128, L, S], f16, kind="ExternalOutput")

    x_flat = x_d.ap().rearrange("c s l -> c (s l)")
    out_flat = out_d.ap().rearrange("c l s -> c (l s)")

    GW = 2048         # spatial columns per slab (16 s-rows)
    NG = SL // GW     # 8
    CH = 16           # i-values per attention chunk
    NCH = 128 // CH   # 8
    NROW_A = 6        # row chunks interleaved into stage A

    # ---------------- top-level tiles ----------------
    with tc.tile_pool(name="base", bufs=1) as base_pool, \
         tc.tile_pool(name="etp", bufs=2) as e_pool, \
         tc.tile_pool(name="srcp", bufs=2) as src_pool, \
         tc.tile_pool(name="denp", bufs=4) as den_pool, \
         tc.tile_pool(name="avr", bufs=2, space="PSUM") as av_pool:
        q2 = base_pool.tile([128, SL], f16, tag="q2")
        k2 = base_pool.tile([128, SL], f16, tag="k2")
        vtr = base_pool.tile([128, SL], f16, tag="vtr")  # [l, hd*128+s]
        vtc = base_pool.tile([128, SL], f16, tag="vtc")  # [s, hd*128+l]
        dst = base_pool.tile([128, SL], f16, tag="dst")  # [hd, s*128+l]
        zero_sb = base_pool.tile([128, 1], f32, tag="z")
        nc.vector.memset(zero_sb[:], 0.0)
        ones_sb = base_pool.tile([128, 1], f16, tag="one")
        nc.vector.memset(ones_sb[:], 1.0)
        ident = base_pool.tile([128, 128], f16, tag="id")
        nc.sync.dma_start(ident[:], id_d.ap())
        bvt_sb = base_pool.tile([128, 512], f32, tag="bvt")
        nc.sync.dma_start(bvt_sb[:], bvt_d.ap())

        qv = q2[:].rearrange("p (s l) -> p s l", l=L)
        kv = k2[:].rearrange("p (s l) -> p s l", l=L)

        def evac_qk(eng, dest_ap, ps_ap, bias_ap):
            if eng == "act":
                nc.scalar.activation(dest_ap, ps_ap, AF.Identity, bias=bias_ap)
            elif eng == "dve":
                nc.vector.tensor_scalar_add(dest_ap, ps_ap, bias_ap)
            else:
                nc.gpsimd.tensor_scalar_add(dest_ap, ps_ap, bias_ap)

        # ---------------- attention chunk ----------------
        # direction 0 = col (i over l), 1 = row (i over s).
        # sc_tile(quad) -> (sc_ap_fn(h, j), exp_fn(ets_out)) abstraction:
        # small mode: per-head [128,512] tiles; big mode: one [128,1024].
        def attention_chunk(direction, ch, vt, sc_alloc):
            i0 = ch * CH

            if direction == 0:
                def qk_slice(t, h, i):
                    return t[h * 64:(h + 1) * 64, :, i]
            else:
                def qk_slice(t, h, i):
                    return t[h * 64:(h + 1) * 64, i, :]

            def vt_slice(h, i):
                return AP(vt.tensor, vt.offset + h * 64 * 128 + i,
                          [list(vt.ap[0]), [128, 64]])

            src = src_pool.tile([128, CH * 128], f16, tag="st", name="src")
            pend = None

            def issue_av(work):
                quad, et_slc, av = work
                for h in range(HPC):
                    for j in range(4):
                        i = i0 + quad * 4 + j
                        nc.tensor.matmul(
                            av[:][:, h, j * 65:j * 65 + 64],
                            et_slc(h, j),
                            vt_slice(h, i),
                            start=True, stop=True)
                        nc.tensor.matmul(
                            av[:][:, h, j * 65 + 64:j * 65 + 65],
                            et_slc(h, j),
                            ones_sb[:][:, 0:1],
                            start=True, stop=True)
                den = den_pool.tile([128, 8], f32, tag="den", name="den")
                nc.vector.reciprocal(
                    den[:], AP(av[:].tensor, av[:].offset + 64,
                               [list(av[:].ap[0]), [65, 4], [512, 2]]))
                in0 = AP(av[:].tensor, av[:].offset,
                         [list(av[:].ap[0]), [65, 4], [512, 2], [1, 64]])
                in1 = AP(den[:].tensor, den[:].offset,
                         [list(den[:].ap[0]), [2, 4], [1, 2], [0, 64]])
                o = AP(src[:].tensor, src[:].offset + (quad * 4) * 128,
                       [list(src[:].ap[0]), [128, 4], [64, 2], [1, 64]])
                nc.vector.tensor_tensor(o, in0, in1, OP.mult)

            for quad in range(4):
                et_slc = sc_alloc(quad, i0, qk_slice)
                av = av_pool.tile([128, 2, 512], f32, tag="av", name="av")
                if pend is not None:
                    issue_av(pend)
                pend = (quad, et_slc, av)
            issue_av(pend)

            if direction == 1:
                # row result straight to dst[hd, (i0+X)*128 + l']
                od = AP(dst[:].tensor, dst[:].offset + i0 * 128,
                        [list(dst[:].ap[0]), [128, CH], [1, 128]])
                nc.sync.dma_start(od, src[:], transpose=True)
            return src

        def small_sc_alloc(pool):
            def alloc(quad, i0, qk_slice):
                ets = []
                for h in range(HPC):
                    sc = pool.tile([128, 512], f32, tag="ps", name="sc")
                    for j in range(4):
                        i = i0 + quad * 4 + j
                        nc.tensor.matmul(
                            sc[:][:, j * 128:(j + 1) * 128],
                            qk_slice(kv, h, i),
                            qk_slice(qv, h, i),
                            start=True, stop=True)
                    et = e_pool.tile([128, 512], f16, tag="et", name="et")
                    nc.scalar.activation(et[:], sc[:], AF.Exp,
                                         bias=zero_sb[:][:, 0:1])
                    ets.append(et)
                return lambda h, j: ets[h][:][:, j * 128:(j + 1) * 128]
            return alloc

        def big_sc_alloc(pool):
            def alloc(quad, i0, qk_slice):
                sc = pool.tile([128, 1024], f32, tag="cs", name="sc")
                for h in range(HPC):
                    for j in range(4):
                        i = i0 + quad * 4 + j
                        nc.tensor.matmul(
                            sc[:][:, h * 512 + j * 128:
                                  h * 512 + (j + 1) * 128],
                            qk_slice(kv, h, i),
                            qk_slice(qv, h, i),
                            start=True, stop=True)
                et = e_pool.tile([128, 1024], f16, tag="et", name="et")
                nc.scalar.activation(et[:], sc[:], AF.Exp,
                                     bias=zero_sb[:][:, 0:1])
                return lambda h, j: et[:][:, h * 512 + j * 128:
                                          h * 512 + (j + 1) * 128]
            return alloc

        # ---- col chunk finish: xbar-transpose into a per-chunk l-major
        # tile, fold the row result in with one strided add, DMA out ----
        def col_transpose(src):
            # src[s', X*128+hd] --xbar--> dc[hd, X*128+s']
            dc = src_pool.tile([128, CH * 128], f16, tag="dc", name="dc",
                               bufs=2)
            ot = AP(dc[:].tensor, dc[:].offset,
                    [list(dc[:].ap[0]), [128, CH], [1, 128]])
            nc.sync.dma_start(ot, src[:], transpose=True)
            return dc

        def col_add(ch, dc, eng):
            # dc[hd, X*128+s'] += dst[hd, s'*128 + (i0+X)]  (row part)
            i0 = ch * CH
            do = AP(dc[:].tensor, dc[:].offset,
                    [list(dc[:].ap[0]), [128, CH], [1, 128]])
            ri = AP(dst[:].tensor, dst[:].offset + i0,
                    [list(dst[:].ap[0]), [1, CH], [128, 128]])
            eng.tensor_tensor(do, do, ri, OP.add)

        def col_out(ch, dc, eng=None):
            i0 = ch * CH
            (eng or nc.sync).dma_start(
                out_flat[:, i0 * 128:(i0 + CH) * 128], dc[:])

        # ---------------- stage A ----------------
        with tc.tile_pool(name="psa", bufs=4, space="PSUM") as psa, \
             tc.tile_pool(name="xload", bufs=2) as xpool:
            small_alloc = small_sc_alloc(psa)
            w_sb = xpool.tile([128, 4, 384], f16, tag="w", bufs=1)
            nc.sync.dma_start(
                w_sb[:], w_d.ap().rearrange("(k c) o -> c k o", k=4))
            b_sb = xpool.tile([128, 3], f32, tag="b", bufs=1)
            nc.sync.dma_start(
                b_sb[:], b_d.ap().rearrange("(m p) -> p m", p=128))
            for g in range(NG):
                xts = []
                for qtr in range(4):
                    xt = xpool.tile([128, 4, GW // 4], f16, tag="x",
                                    bufs=5, name="xt")
                    lo = g * GW + qtr * (GW // 4)
                    nc.sync.dma_start(
                        xt[:],
                        x_flat[:, lo:lo + GW // 4]
                            .rearrange("(k c) n -> c k n", k=4))
                    xts.append(xt)
                # q, k projections: W stationary, x moving
                for m in range(2):
                    dest = (q2, k2)[m]
                    eng_rot = ("act", "act", "act", "act")
                    for sg in range(4):
                        ps = psa.tile([128, 512], f32, tag="ps", name="ps")
                        for c in range(4):
                            nc.tensor.matmul(
                                ps[:],
                                w_sb[:][:, c, m * 128:(m + 1) * 128],
                                xts[sg][:][:, c, :],
                                start=(c == 0), stop=(c == 3))
                        off = g * GW + sg * 512
                        evac_qk(eng_rot[sg], dest[:][:, off:off + 512],
                                ps[:], b_sb[:][:, m:m + 1])
                # v projection flipped: x-tile stationary, Wv^T moving
                for sg in range(4):
                    vt_ps = psa.tile([128, 512], f32, tag="ps", name="vt")
                    for sr in range(4):
                        for c in range(4):
                            nc.tensor.matmul(
                                vt_ps[:][:, sr * 128:(sr + 1) * 128],
                                xts[sg][:][:, c, sr * 128:(sr + 1) * 128],
                                w_sb[:][:, c, 256:384],
                                start=(c == 0), stop=(c == 3))
                    # vtr[l, hd*128 + (s0+j)] = vt_ps[l, j*128+hd] + b_v
                    s0 = g * 16 + sg * 4
                    vo = AP(vtr[:].tensor, vtr[:].offset + s0,
                            [list(vtr[:].ap[0]), [128, 128], [1, 4]])
                    vi = AP(vt_ps[:].tensor, vt_ps[:].offset,
                            [list(vt_ps[:].ap[0]), [1, 128], [128, 4]])
                    bi = AP(bvt_sb[:].tensor, bvt_sb[:].offset,
                            [list(bvt_sb[:].ap[0]), [1, 128], [128, 4]])
                    nc.vector.tensor_tensor(vo, vi, bi, OP.add)
                import os
                if g < NROW_A and os.environ.get("AXIAL_STAGE") != "col":
                    attention_chunk(1, g, vtr, small_alloc)

        # ---------------- col phase ----------------
        import os
        stage = os.environ.get("AXIAL_STAGE", "full")
        with tc.tile_pool(name="psb", bufs=2, space="PSUM") as psb:
            big_alloc = big_sc_alloc(psb)
            # vtc[s, (16t+a)*128 + l] <- vtr[l, (16t+a)*128 + s]
            def vtc_issue(ts):
                for t in ts:
                    vo = AP(vtc[:].tensor, vtc[:].offset + t * 2048,
                            [list(vtc[:].ap[0]), [128, 16], [1, 128]])
                    nc.sync.dma_start(vo,
                                      vtr[:][:, t * 2048:(t + 1) * 2048],
                                      transpose=True)
            vtc_issue(range(0, 8))
            # remaining row chunks fill the vtc-transpose window
            if stage == "col":
                nc.vector.memset(dst[:], 0.0)
            else:
                for g in range(NROW_A, NG):
                    attention_chunk(1, g, vtr, big_alloc)
            if stage == "row":
                for sb in range(8):
                    nc.sync.dma_start(
                        out_flat[:, sb * 2048:(sb + 1) * 2048],
                        dst[:][:, sb * 2048:(sb + 1) * 2048])
                return
            # software-pipelined finish, staged so every op's wait is
            # short when its queue reaches it: trC(ch-1) + Pool-half
            # merge(ch-1) behind chunk ch; DVE-half merge(ch-2) (its trC
            # is certainly done -> the DVE queue never waits on DMA);
            # out(ch-3) on SP.
            srcs, dcs = {}, {}
            LAST = NCH - 1  # last chunk finishes eagerly (DVE add)

            def finish_steps(ch):
                if 1 <= ch and ch - 1 < LAST and (ch - 1) not in dcs:
                    dcs[ch - 1] = col_transpose(srcs[ch - 1])
                if 2 <= ch and ch - 2 < LAST:
                    col_add(ch - 2, dcs[ch - 2], nc.vector)
                if 3 <= ch and ch - 3 < LAST:
                    col_out(ch - 3, dcs[ch - 3])

            for ch in range(NCH):
                srcs[ch] = attention_chunk(0, ch, vtc, big_alloc)
                finish_steps(ch)
                if ch >= LAST:
                    # eager tail: transpose -> Pool add -> ACT-issued out
                    dcs[ch] = col_transpose(srcs[ch])
                    col_add(ch, dcs[ch], nc.vector)
                    col_out(ch, dcs[ch], nc.scalar)
            for ch in range(NCH, NCH + 3):
                finish_steps(ch)


def _get_nc():
    if "nc" in _CACHE:
        return _CACHE["nc"]
    import concourse.bacc as bacc
    import concourse.tile as tile

    nc = bacc.Bacc(None, target_bir_lowering=False, debug=False,
                   num_devices=N_CORES)
    with tile.TileContext(nc) as tc:
        build_program(nc, tc)
    nc.compile()
    _CACHE["nc"] = nc
    return nc


def make_in_maps(x, W, b):
    x = np.asarray(x, dtype=np.float32)
    W = np.asarray(W, dtype=np.float32)
    b = np.asarray(b, dtype=np.float32)
    scale = np.float32(DIM_HEAD ** -0.5)
    ident = np.eye(128, dtype=np.float16)
    in_maps = []
    for c in range(N_CORES):
        bb, h0 = c // 4, 2 * (c % 4)
        hd = np.arange(h0 * 64, (h0 + 2) * 64)
        sel = np.concatenate([hd, EMBED + hd, 2 * EMBED + hd])
        W_loc = W[sel, :].copy()
        b_loc = b[sel].copy()
        W_loc[:128] *= scale
        b_loc[:128] *= scale
        bvt = np.tile(b_loc[256:384], 4)[None, :].repeat(128, 0)
        in_maps.append({
            "x": np.ascontiguousarray(x[bb]).astype(np.float16),
            "wT": np.ascontiguousarray(W_loc.T).astype(np.float16),
            "bvec": b_loc.astype(np.float32),
            "bvt": np.ascontiguousarray(bvt).astype(np.float32),
            "ident": ident,
        })
    return in_maps


def assemble(results):
    out = np.empty((B, EMBED, S, L), dtype=np.float32)
    for c, r in enumerate(results):
        bb, h0 = c // 4, 2 * (c % 4)
        # device output is [hd, l, s]; swap back to [hd, s, l]
        out[bb, h0 * 64:(h0 + 2) * 64] = \
            np.swapaxes(r["out"], 1, 2).astype(np.float32)
    return out


def kernel(x, W, b):
    from concourse.bass_utils import run_bass_kernel_spmd
    nc = _get_nc()
    res = run_bass_kernel_spmd(nc, make_in_maps(x, W, b),
                               core_ids=list(range(N_CORES)))
    return assemble(res.results)
